# revision 18
# baseline (speedup 1.0000x reference)
"""Trainium2 Bass kernel for a BCE-based decoding loss (v3).

Math: with t = tanh(llrs/2),
  p[b,r]   = clip(prod_w t[b, idx[r,w]], -1+eps, 1-eps)
  bce(z,y) = softplus(z) - z*y  with  z = -2*arctanh(p)
which for y in {0,1} simplifies exactly to
  bce = log(2) - log(1 + (1-2y) * p)
so   loss = 0.5*(M+K)*log(2) - (0.5/B) * sum_{b,r} log(1 + s[b,r]*p[b,r])
with s = 1-2y.  (The clip never binds on this data: min(1+sp) = 0.33.)

Sharding: pure data parallel over batch -- 8 cores x 128 rows each.

Host-side prep (data movement / quantization only, no transcendental
math): llrs are halved+clipped+scaled (z' = alpha*clip(llr/2, +-Z)),
cast to fp8 e4m3, and gathered per (check, w) slot.  The label
s = (1-2y) is folded into the SIGN BIT of the w=0 slot (tanh is odd).
Slot layout per check tile pairs slot A (w<4) with slot B (w>=4) so
the first product-tree level can be fused into the tanh of the B half.

Device math, per check tile (C checks, S = 8*C slots), measured HW
rates in ns/elem/partition in brackets:
  tA = tanh(g[:, 0:S/2] / alpha)           ACT [0.89], exact, fp8 in
  h  = TANH5M(g[:, S/2:S]) * tA            custom DVE [1.08]: deg-5
       odd poly of tanh times tA -> fused level-1 products
  ('A' tiles: ACT does exact tanh on ALL slots; level-1 is a DVE
   bf16 tensor_mul [0.59] -- this trades DVE mul work for custom-op
   work to balance the two engines)
  q  = h0 * h1                             level-2 DVE mul [0.59]
  LOG1PM(q0, q1) -> acc                    custom DVE: ln(1+a*b) deg-3
       series with accum_out = per-partition row sum (fuses level-3,
       the log, and the reduction)
The observable tile (8 obs x 128 slots, fused halves + 5 small muls +
LOG1PM) is emitted early so its serial chain hides under check tiles.
ACT and DVE each run ~48 us busy and overlap gaplessly; fp8 DMA
(~30-36 us) stays underneath.  GpSimd/Pool gets NO tensor work: its
software muls monopolize SBUF bandwidth and halve concurrent DVE/ACT
throughput (measured; see v3 note below).
"""

import math
import os

import numpy as np

os.environ.setdefault("MYCRO_LOCAL_CACHE", "1")

import ml_dtypes  # noqa: E402

B, N, M, K = 1024, 16384, 8192, 8
WC, WO = 8, 128
NCORES = 8
BL = B // NCORES            # batch rows per core = 128
OBS_SLOTS = K * WO          # 1024 obs slots
TOT_SLOTS = M * WC + OBS_SLOTS       # 66560
EPS = 1e-6

# Tile plan: (checks, kind); kind 'F' = fused (ACT half + custom half),
# 'A' = ACT-all (exact tanh everywhere, level-1 is a DVE mul).  Two small
# starter tiles let compute begin ~1.5us earlier; the ratio (3072 F /
# 5120 A checks) balances ACT (~48us) against DVE (~46us).
TILE_PLAN = [(512, "A"), (256, "F"), (256, "F"), (512, "F"),
             (1024, "A"), (1024, "F"), (1024, "A"), (1024, "F"),
             (1024, "A"), (1024, "A"), (512, "A")]
assert sum(c for c, _ in TILE_PLAN) == M
NTILES = len(TILE_PLAN)

# tanh(z) ~= z'(PA + u(PB + u)), z' = ALPHA*z, u = z'^2, for |z| <= ZCLIP
PA, PB, ALPHA, ZCLIP = 2.331135, -2.379626, 0.374835, 3.0
# ln(1+y) ~= y + y^2*(L0 + y*L1)
L0, L1_ = -0.5, 1.0 / 3.0

# Pool/GpSimd is deliberately UNUSED for tensor work: its software muls
# monopolize SBUF bandwidth and halve DVE+ACT throughput while active
# (measured v3: every DVE/ACT instr overlapping a Pool mul ran ~2x slow).
NACC = NTILES + 1           # accumulator columns (check tiles + obs)

_CACHE = {}


def _register_custom_ops():
    """Register the kernel-specific custom DVE ops with the dve_ops
    registry (the documented extension point is appending to OPS; doing it
    at runtime keeps kernel.py self-contained).  Pins uops_sha from the
    actual lowering so DveOp.compile's drift check passes."""
    if "ops" in _CACHE:
        return _CACHE["ops"]
    import concourse.dve_ops as dve_ops
    from concourse.dve_spec import (
        Spec, Src0, Src1, C0, C1, Zero, sq, lower, _has_src1,
    )
    from concourse.dve_uop import DveOpSpec
    from operator import add

    u = sq(Src0)
    body_t = (Src0 * (C0 + u * (C1 + u))) * Src1

    def ref_t(in0, in1, s0, s1, imm2):
        x = in0.astype(np.float32)
        uu = x * x
        return (x * (s0 + uu * (s1 + uu)) * in1.astype(np.float32)).astype(
            np.float32)

    tanh5m = dve_ops.DveOp(
        "TANH5M_ANT", Spec(body=body_t, reference=ref_t),
        subdim=False, uops_sha={})

    m = Src0 * Src1
    body_l = m + sq(m) * (C0 + m * C1)

    def ref_l(in0, in1, c0, c1, c2):
        y = in0.astype(np.float32) * in1.astype(np.float32)
        b = (y + y * y * (c0 + y * c1)).astype(np.float32)
        return b, b.reshape(b.shape[0], -1).sum(axis=-1, keepdims=True)

    log1pm = dve_ops.DveOp(
        "LOG1PM_ANT",
        Spec(body=body_l, accum=add, accum_init=Zero, reference=ref_l),
        subdim=False, uops_sha={})

    for op in (tanh5m, log1pm):
        if op.name not in dve_ops._SUB_OPCODE_FOR_NAME:
            dve_ops.OPS.append(op)
            dve_ops.CUSTOM_DVE_SPECS[op.name] = op.spec
            dve_ops._SUB_OPCODE_FOR_NAME[op.name] = (
                dve_ops._CUSTOM_DVE_ROW_BASE + len(dve_ops.OPS) - 1)
        shas = {}
        for ver in ("v3", "v4"):
            spec = DveOpSpec(
                name=op.name,
                opcode=dve_ops.get_dve_sub_opcode(op.name),
                uops=lower(op.spec, ver=ver),
                rd1_en=_has_src1(op.spec),
            )
            shas[ver] = spec.sha(ver)
        object.__setattr__(op, "uops_sha", shas)
    _CACHE["ops"] = (tanh5m, log1pm)
    return _CACHE["ops"]


def build_nc():
    import concourse.bacc as bacc
    import concourse.mybir as mybir
    import concourse.tile as tile
    from contextlib import ExitStack

    tanh5m, log1pm = _register_custom_ops()

    nc = bacc.Bacc("TRN2", target_bir_lowering=False, debug=False)
    f32 = mybir.dt.float32
    bf16 = mybir.dt.bfloat16
    f8 = mybir.dt.float8e4

    g_dram = nc.dram_tensor("g", [BL, TOT_SLOTS], f8, kind="ExternalInput")
    out = nc.dram_tensor("out", [128, NACC], f32, kind="ExternalOutput")

    Tanh = mybir.ActivationFunctionType.Tanh

    with tile.TileContext(nc) as tc:
        with ExitStack() as ctx:
            singles = ctx.enter_context(tc.tile_pool(name="singles", bufs=1))
            gp = ctx.enter_context(tc.tile_pool(name="gp", bufs=3))
            tp = ctx.enter_context(tc.tile_pool(name="tp", bufs=2))
            hp = ctx.enter_context(tc.tile_pool(name="hp", bufs=2))
            qp = ctx.enter_context(tc.tile_pool(name="qp", bufs=2))

            acc = singles.tile([128, NACC], f32)

            def check_tile(t, off, ct, kind):
                ts_ = ct * WC
                g = gp.tile([128, ts_], f8, tag=f"g{ts_}")
                nc.sync.dma_start(g[:], g_dram[:, off:off + ts_])
                h = hp.tile([128, ts_ // 2], bf16, tag=f"h{ts_}")
                if kind == "F":
                    tA = tp.tile([128, ts_ // 2], bf16, tag=f"tA{ts_}")
                    nc.scalar.activation(tA[:], g[:, 0:ts_ // 2], Tanh,
                                         bias=0.0, scale=1.0 / ALPHA)
                    nc.vector._custom_dve(
                        tanh5m, out=h[:], in0=g[:, ts_ // 2:ts_], in1=tA[:],
                        s0=PA, s1=PB)
                else:
                    tA = tp.tile([128, ts_], bf16, tag=f"tF{ts_}")
                    nc.scalar.activation(tA[:], g[:], Tanh,
                                         bias=0.0, scale=1.0 / ALPHA)
                    nc.vector.tensor_mul(h[:], tA[:, 0:ts_ // 2],
                                         tA[:, ts_ // 2:ts_])
                q = qp.tile([128, ts_ // 4], bf16, tag=f"q{ts_}")
                nc.vector.tensor_mul(q[:], h[:, 0:ts_ // 4],
                                     h[:, ts_ // 4:ts_ // 2])
                lnj = qp.tile([128, ct], bf16, tag=f"lnj{ts_}")
                nc.vector._custom_dve(
                    log1pm, out=lnj[:], in0=q[:, 0:ct], in1=q[:, ct:2 * ct],
                    s0=L0, s1=L1_, accum_out=acc[:, t:t + 1])

            def obs_tile():
                ob = M * WC
                g = gp.tile([128, OBS_SLOTS], f8, tag="go")
                nc.sync.dma_start(g[:], g_dram[:, ob:ob + OBS_SLOTS])
                tA = tp.tile([128, 512], bf16, tag="tAo")
                nc.scalar.activation(tA[:], g[:, 0:512], Tanh,
                                     bias=0.0, scale=1.0 / ALPHA)
                h = hp.tile([128, 512], bf16, tag="ho")
                nc.vector._custom_dve(
                    tanh5m, out=h[:], in0=g[:, 512:1024], in1=tA[:],
                    s0=PA, s1=PB)
                w = 512
                while w > 2 * K:
                    w //= 2
                    nh = qp.tile([128, w], bf16, tag=f"o{w}")
                    nc.vector.tensor_mul(nh[:], h[:, 0:w], h[:, w:2 * w])
                    h = nh
                lno = qp.tile([128, K], bf16, tag="lno")
                nc.vector._custom_dve(
                    log1pm, out=lno[:], in0=h[:, 0:K], in1=h[:, K:2 * K],
                    s0=L0, s1=L1_, accum_out=acc[:, NTILES:NTILES + 1])

            offs = np.cumsum([0] + [c * WC for c, _ in TILE_PLAN])
            check_tile(0, int(offs[0]), *TILE_PLAN[0])
            obs_tile()
            for t in range(1, NTILES):
                check_tile(t, int(offs[t]), *TILE_PLAN[t])

            nc.sync.dma_start(out[:, :], acc[:])

    nc.compile()
    return nc


def get_nc():
    if "nc" not in _CACHE:
        _CACHE["nc"] = build_nc()
    return _CACHE["nc"]


def build_slots(chk_idx, obs_idx):
    """Column j of the shipped tensor holds z'[idx] for slot-order:
    check tile t (ct checks at check-offset r0, slot-offset o = 8*r0):
                  A-half j = o + v*ct + c      -> chk[r0+c, v]
                  B-half j = o + 4*ct + v*ct + c -> chk[r0+c, v+4]
    obs tile:     A-half j = M*WC + v*K + k      -> obs[k, v]
                  B-half j = M*WC + 512 + v*K + k -> obs[k, v+64]
    so every tree level multiplies contiguous halves."""
    chk = np.asarray(chk_idx)
    obs = np.asarray(obs_idx)
    parts = []
    r0 = 0
    for ct, _ in TILE_PLAN:
        sub = chk[r0:r0 + ct]                           # [ct, 8]
        parts.append(sub[:, 0:4].T.reshape(-1))         # v-major A half
        parts.append(sub[:, 4:8].T.reshape(-1))         # v-major B half
        r0 += ct
    parts.append(obs[:, 0:WO // 2].T.reshape(-1))
    parts.append(obs[:, WO // 2:WO].T.reshape(-1))
    return np.concatenate(parts).astype(np.int64)


def make_in_maps(llrs, syndromes, observables, chk_idx, obs_idx):
    zp = (np.clip(np.asarray(llrs) * 0.5, -ZCLIP, ZCLIP) * ALPHA).astype(
        ml_dtypes.float8_e4m3)
    slots = build_slots(chk_idx, obs_idx)
    g_all = np.take(zp, slots, axis=1)                  # [B, TOT_SLOTS]
    # fold s = (1-2y) into the sign bit of the w=0 slot of each check
    v = g_all.view(np.uint8)
    syn = np.asarray(syndromes)
    r0 = 0
    for ct, _ in TILE_PLAN:
        cols = slice(8 * r0, 8 * r0 + ct)               # v=0 of A half
        v[:, cols] ^= (syn[:, r0:r0 + ct] != 0).astype(np.uint8) << 7
        r0 += ct
    yobs = (np.asarray(observables) != 0).astype(np.uint8) << 7
    v[:, M * WC:M * WC + K] ^= yobs                     # v=0 of obs A half
    return [{"g": g_all[BL * c:BL * (c + 1)]} for c in range(NCORES)]


def finish(results):
    total = 0.0
    for r in results:
        total += float(np.asarray(r["out"]).astype(np.float64).sum())
    loss = 0.5 * (M + K) * math.log(2.0) - 0.5 * total / B
    return np.float32(loss)


def kernel(llrs, syndromes, observables, chk_idx, obs_idx):
    from concourse.bass_utils import run_bass_kernel_spmd

    in_maps = make_in_maps(llrs, syndromes, observables, chk_idx, obs_idx)
    nc = get_nc()
    res = run_bass_kernel_spmd(nc, in_maps, core_ids=list(range(NCORES)))
    return finish(res.results)


# revision 21
# speedup vs baseline: 1.0571x; 1.0571x over previous
"""Trainium2 Bass kernel for a BCE-based decoding loss (v3).

Math: with t = tanh(llrs/2),
  p[b,r]   = clip(prod_w t[b, idx[r,w]], -1+eps, 1-eps)
  bce(z,y) = softplus(z) - z*y  with  z = -2*arctanh(p)
which for y in {0,1} simplifies exactly to
  bce = log(2) - log(1 + (1-2y) * p)
so   loss = 0.5*(M+K)*log(2) - (0.5/B) * sum_{b,r} log(1 + s[b,r]*p[b,r])
with s = 1-2y.  (The clip never binds on this data: min(1+sp) = 0.33.)

Sharding: pure data parallel over batch -- 8 cores x 128 rows each.

Host-side prep (data movement / quantization only, no transcendental
math): llrs are halved+clipped+scaled (z' = alpha*clip(llr/2, +-Z)),
cast to fp8 e4m3, and gathered per (check, w) slot.  The label
s = (1-2y) is folded into the SIGN BIT of the w=0 slot (tanh is odd).
Slot layout per check tile pairs slot A (w<4) with slot B (w>=4) so
the first product-tree level can be fused into the tanh of the B half.

Device math, per check tile (C checks, S = 8*C slots), measured HW
rates in ns/elem/partition in brackets:
  tA = tanh(g[:, 0:S/2] / alpha)           ACT [0.89], exact, fp8 in
  h  = TANH5M(g[:, S/2:S]) * tA            custom DVE [1.08]: deg-5
       odd poly of tanh times tA -> fused level-1 products
  ('A' tiles: ACT does exact tanh on ALL slots; level-1 is a DVE
   bf16 tensor_mul [0.59] -- this trades DVE mul work for custom-op
   work to balance the two engines)
  q  = h0 * h1                             level-2 DVE mul [0.59]
  LOG1PM(q0, q1) -> acc                    custom DVE: ln(1+a*b) deg-3
       series with accum_out = per-partition row sum (fuses level-3,
       the log, and the reduction)
The observable tile (8 obs x 128 slots, fused halves + 5 small muls +
LOG1PM) is emitted early so its serial chain hides under check tiles.
ACT and DVE each run ~48 us busy and overlap gaplessly; fp8 DMA
(~30-36 us) stays underneath.  GpSimd/Pool gets NO tensor work: its
software muls monopolize SBUF bandwidth and halve concurrent DVE/ACT
throughput (measured; see v3 note below).
"""

import math
import os

import numpy as np

os.environ.setdefault("MYCRO_LOCAL_CACHE", "1")

import ml_dtypes  # noqa: E402

B, N, M, K = 1024, 16384, 8192, 8
WC, WO = 8, 128
NCORES = 8
BL = B // NCORES            # batch rows per core = 128
OBS_SLOTS = K * WO          # 1024 obs slots
TOT_SLOTS = M * WC + OBS_SLOTS       # 66560
EPS = 1e-6

# Tile plan: (checks, kind); kind 'F' = fused (ACT half + custom half),
# 'A' = ACT-all (exact tanh everywhere, level-1 is a DVE mul).  Two small
# starter tiles let compute begin ~1.5us earlier; the ratio (3072 F /
# 5120 A checks) balances ACT (~48us) against DVE (~46us).
TILE_PLAN = [(256, "F"), (256, "F"), (512, "F"), (1024, "A"),
             (1024, "F"), (1024, "A"), (1024, "F"), (1024, "A"),
             (1024, "A"), (512, "A"), (512, "A")]
OBS_AFTER = 2               # emit the obs tile after this many check tiles:
                            # its DVE chain fills the Vector stall that
                            # otherwise opens during the first big A-tile's
                            # 7.3us ACT activation
assert sum(c for c, _ in TILE_PLAN) == M
NTILES = len(TILE_PLAN)

# tanh(z) ~= z'(PA + u(PB + u)), z' = ALPHA*z, u = z'^2, for |z| <= ZCLIP
PA, PB, ALPHA, ZCLIP = 2.331135, -2.379626, 0.374835, 3.0
# ln(1+y) ~= y + y^2*(L0 + y*L1)
L0, L1_ = -0.5, 1.0 / 3.0

# Pool/GpSimd is deliberately UNUSED for tensor work: its software muls
# monopolize SBUF bandwidth and halve DVE+ACT throughput while active
# (measured v3: every DVE/ACT instr overlapping a Pool mul ran ~2x slow).
NACC = NTILES + 1           # accumulator columns (check tiles + obs)

_CACHE = {}


def _register_custom_ops():
    """Register the kernel-specific custom DVE ops with the dve_ops
    registry (the documented extension point is appending to OPS; doing it
    at runtime keeps kernel.py self-contained).  Pins uops_sha from the
    actual lowering so DveOp.compile's drift check passes."""
    if "ops" in _CACHE:
        return _CACHE["ops"]
    import concourse.dve_ops as dve_ops
    from concourse.dve_spec import (
        Spec, Src0, Src1, C0, C1, Zero, sq, lower, _has_src1,
    )
    from concourse.dve_uop import DveOpSpec
    from operator import add

    u = sq(Src0)
    body_t = (Src0 * (C0 + u * (C1 + u))) * Src1

    def ref_t(in0, in1, s0, s1, imm2):
        x = in0.astype(np.float32)
        uu = x * x
        return (x * (s0 + uu * (s1 + uu)) * in1.astype(np.float32)).astype(
            np.float32)

    tanh5m = dve_ops.DveOp(
        "TANH5M_ANT", Spec(body=body_t, reference=ref_t),
        subdim=False, uops_sha={})

    m = Src0 * Src1
    body_l = m + sq(m) * (C0 + m * C1)

    def ref_l(in0, in1, c0, c1, c2):
        y = in0.astype(np.float32) * in1.astype(np.float32)
        b = (y + y * y * (c0 + y * c1)).astype(np.float32)
        return b, b.reshape(b.shape[0], -1).sum(axis=-1, keepdims=True)

    log1pm = dve_ops.DveOp(
        "LOG1PM_ANT",
        Spec(body=body_l, accum=add, accum_init=Zero, reference=ref_l),
        subdim=False, uops_sha={})

    for op in (tanh5m, log1pm):
        if op.name not in dve_ops._SUB_OPCODE_FOR_NAME:
            dve_ops.OPS.append(op)
            dve_ops.CUSTOM_DVE_SPECS[op.name] = op.spec
            dve_ops._SUB_OPCODE_FOR_NAME[op.name] = (
                dve_ops._CUSTOM_DVE_ROW_BASE + len(dve_ops.OPS) - 1)
        shas = {}
        for ver in ("v3", "v4"):
            spec = DveOpSpec(
                name=op.name,
                opcode=dve_ops.get_dve_sub_opcode(op.name),
                uops=lower(op.spec, ver=ver),
                rd1_en=_has_src1(op.spec),
            )
            shas[ver] = spec.sha(ver)
        object.__setattr__(op, "uops_sha", shas)
    _CACHE["ops"] = (tanh5m, log1pm)
    return _CACHE["ops"]


def build_nc():
    import concourse.bacc as bacc
    import concourse.mybir as mybir
    import concourse.tile as tile
    from contextlib import ExitStack

    tanh5m, log1pm = _register_custom_ops()

    nc = bacc.Bacc("TRN2", target_bir_lowering=False, debug=False)
    f32 = mybir.dt.float32
    bf16 = mybir.dt.bfloat16
    f8 = mybir.dt.float8e4

    g_dram = nc.dram_tensor("g", [BL, TOT_SLOTS], f8, kind="ExternalInput")
    out = nc.dram_tensor("out", [128, NACC], f32, kind="ExternalOutput")

    Tanh = mybir.ActivationFunctionType.Tanh

    with tile.TileContext(nc) as tc:
        with ExitStack() as ctx:
            singles = ctx.enter_context(tc.tile_pool(name="singles", bufs=1))
            gp = ctx.enter_context(tc.tile_pool(name="gp", bufs=4))
            tp = ctx.enter_context(tc.tile_pool(name="tp", bufs=2))
            hp = ctx.enter_context(tc.tile_pool(name="hp", bufs=2))
            qp = ctx.enter_context(tc.tile_pool(name="qp", bufs=2))

            acc = singles.tile([128, NACC], f32)

            def check_tile(t, off, ct, kind):
                ts_ = ct * WC
                g = gp.tile([128, ts_], f8, tag=f"g{ts_}")
                nc.sync.dma_start(g[:], g_dram[:, off:off + ts_])
                h = hp.tile([128, ts_ // 2], bf16, tag=f"h{ts_}")
                if kind == "F":
                    tA = tp.tile([128, ts_ // 2], bf16, tag=f"tA{ts_}")
                    nc.scalar.activation(tA[:], g[:, 0:ts_ // 2], Tanh,
                                         bias=0.0, scale=1.0 / ALPHA)
                    nc.vector._custom_dve(
                        tanh5m, out=h[:], in0=g[:, ts_ // 2:ts_], in1=tA[:],
                        s0=PA, s1=PB)
                else:
                    tA = tp.tile([128, ts_], bf16, tag=f"tF{ts_}")
                    nc.scalar.activation(tA[:], g[:], Tanh,
                                         bias=0.0, scale=1.0 / ALPHA)
                    nc.vector.tensor_mul(h[:], tA[:, 0:ts_ // 2],
                                         tA[:, ts_ // 2:ts_])
                q = qp.tile([128, ts_ // 4], bf16, tag=f"q{ts_}")
                nc.vector.tensor_mul(q[:], h[:, 0:ts_ // 4],
                                     h[:, ts_ // 4:ts_ // 2])
                lnj = qp.tile([128, ct], bf16, tag=f"lnj{ts_}")
                nc.vector._custom_dve(
                    log1pm, out=lnj[:], in0=q[:, 0:ct], in1=q[:, ct:2 * ct],
                    s0=L0, s1=L1_, accum_out=acc[:, t:t + 1])

            def obs_tile():
                ob = M * WC
                g = gp.tile([128, OBS_SLOTS], f8, tag="go")
                nc.sync.dma_start(g[:], g_dram[:, ob:ob + OBS_SLOTS])
                tA = tp.tile([128, 512], bf16, tag="tAo")
                nc.scalar.activation(tA[:], g[:, 0:512], Tanh,
                                     bias=0.0, scale=1.0 / ALPHA)
                h = hp.tile([128, 512], bf16, tag="ho")
                nc.vector._custom_dve(
                    tanh5m, out=h[:], in0=g[:, 512:1024], in1=tA[:],
                    s0=PA, s1=PB)
                w = 512
                while w > 2 * K:
                    w //= 2
                    nh = qp.tile([128, w], bf16, tag=f"o{w}")
                    nc.vector.tensor_mul(nh[:], h[:, 0:w], h[:, w:2 * w])
                    h = nh
                lno = qp.tile([128, K], bf16, tag="lno")
                nc.vector._custom_dve(
                    log1pm, out=lno[:], in0=h[:, 0:K], in1=h[:, K:2 * K],
                    s0=L0, s1=L1_, accum_out=acc[:, NTILES:NTILES + 1])

            offs = np.cumsum([0] + [c * WC for c, _ in TILE_PLAN])
            for t in range(NTILES):
                check_tile(t, int(offs[t]), *TILE_PLAN[t])
                if t == OBS_AFTER:
                    obs_tile()

            nc.sync.dma_start(out[:, :], acc[:])

    nc.compile()
    return nc


def get_nc():
    if "nc" not in _CACHE:
        _CACHE["nc"] = build_nc()
    return _CACHE["nc"]


def build_slots(chk_idx, obs_idx):
    """Column j of the shipped tensor holds z'[idx] for slot-order:
    check tile t (ct checks at check-offset r0, slot-offset o = 8*r0):
                  A-half j = o + v*ct + c      -> chk[r0+c, v]
                  B-half j = o + 4*ct + v*ct + c -> chk[r0+c, v+4]
    obs tile:     A-half j = M*WC + v*K + k      -> obs[k, v]
                  B-half j = M*WC + 512 + v*K + k -> obs[k, v+64]
    so every tree level multiplies contiguous halves."""
    chk = np.asarray(chk_idx)
    obs = np.asarray(obs_idx)
    parts = []
    r0 = 0
    for ct, _ in TILE_PLAN:
        sub = chk[r0:r0 + ct]                           # [ct, 8]
        parts.append(sub[:, 0:4].T.reshape(-1))         # v-major A half
        parts.append(sub[:, 4:8].T.reshape(-1))         # v-major B half
        r0 += ct
    parts.append(obs[:, 0:WO // 2].T.reshape(-1))
    parts.append(obs[:, WO // 2:WO].T.reshape(-1))
    return np.concatenate(parts).astype(np.int64)


def make_in_maps(llrs, syndromes, observables, chk_idx, obs_idx):
    zp = (np.clip(np.asarray(llrs) * 0.5, -ZCLIP, ZCLIP) * ALPHA).astype(
        ml_dtypes.float8_e4m3)
    slots = build_slots(chk_idx, obs_idx)
    g_all = np.take(zp, slots, axis=1)                  # [B, TOT_SLOTS]
    # fold s = (1-2y) into the sign bit of the w=0 slot of each check
    v = g_all.view(np.uint8)
    syn = np.asarray(syndromes)
    r0 = 0
    for ct, _ in TILE_PLAN:
        cols = slice(8 * r0, 8 * r0 + ct)               # v=0 of A half
        v[:, cols] ^= (syn[:, r0:r0 + ct] != 0).astype(np.uint8) << 7
        r0 += ct
    yobs = (np.asarray(observables) != 0).astype(np.uint8) << 7
    v[:, M * WC:M * WC + K] ^= yobs                     # v=0 of obs A half
    return [{"g": g_all[BL * c:BL * (c + 1)]} for c in range(NCORES)]


def finish(results):
    total = 0.0
    for r in results:
        total += float(np.asarray(r["out"]).astype(np.float64).sum())
    loss = 0.5 * (M + K) * math.log(2.0) - 0.5 * total / B
    return np.float32(loss)


def kernel(llrs, syndromes, observables, chk_idx, obs_idx):
    from concourse.bass_utils import run_bass_kernel_spmd

    in_maps = make_in_maps(llrs, syndromes, observables, chk_idx, obs_idx)
    nc = get_nc()
    res = run_bass_kernel_spmd(nc, in_maps, core_ids=list(range(NCORES)))
    return finish(res.results)


# revision 22
# speedup vs baseline: 1.0714x; 1.0135x over previous
"""Trainium2 Bass kernel for a BCE-based decoding loss (v3).

Math: with t = tanh(llrs/2),
  p[b,r]   = clip(prod_w t[b, idx[r,w]], -1+eps, 1-eps)
  bce(z,y) = softplus(z) - z*y  with  z = -2*arctanh(p)
which for y in {0,1} simplifies exactly to
  bce = log(2) - log(1 + (1-2y) * p)
so   loss = 0.5*(M+K)*log(2) - (0.5/B) * sum_{b,r} log(1 + s[b,r]*p[b,r])
with s = 1-2y.  (The clip never binds on this data: min(1+sp) = 0.33.)

Sharding: pure data parallel over batch -- 8 cores x 128 rows each.

Host-side prep (data movement / quantization only, no transcendental
math): llrs are halved+clipped+scaled (z' = alpha*clip(llr/2, +-Z)),
cast to fp8 e4m3, and gathered per (check, w) slot.  The label
s = (1-2y) is folded into the SIGN BIT of the w=0 slot (tanh is odd).
Slot layout per check tile pairs slot A (w<4) with slot B (w>=4) so
the first product-tree level can be fused into the tanh of the B half.

Device math, per check tile (C checks, S = 8*C slots), measured HW
rates in ns/elem/partition in brackets:
  tA = tanh(g[:, 0:S/2] / alpha)           ACT [0.89], exact, fp8 in
  h  = TANH5M(g[:, S/2:S]) * tA            custom DVE [1.08]: deg-5
       odd poly of tanh times tA -> fused level-1 products
  ('A' tiles: ACT does exact tanh on ALL slots; level-1 is a DVE
   bf16 tensor_mul [0.59] -- this trades DVE mul work for custom-op
   work to balance the two engines)
  q  = h0 * h1                             level-2 DVE mul [0.59]
  LOG1PM(q0, q1) -> acc                    custom DVE: ln(1+a*b) deg-3
       series with accum_out = per-partition row sum (fuses level-3,
       the log, and the reduction)
The observable tile (8 obs x 128 slots, fused halves + 5 small muls +
LOG1PM) is emitted early so its serial chain hides under check tiles.
ACT and DVE each run ~48 us busy and overlap gaplessly; fp8 DMA
(~30-36 us) stays underneath.  GpSimd/Pool gets NO tensor work: its
software muls monopolize SBUF bandwidth and halve concurrent DVE/ACT
throughput (measured; see v3 note below).
"""

import math
import os

import numpy as np

os.environ.setdefault("MYCRO_LOCAL_CACHE", "1")

import ml_dtypes  # noqa: E402

B, N, M, K = 1024, 16384, 8192, 8
WC, WO = 8, 128
NCORES = 8
BL = B // NCORES            # batch rows per core = 128
OBS_SLOTS = K * WO          # 1024 obs slots
TOT_SLOTS = M * WC + OBS_SLOTS       # 66560
EPS = 1e-6

# Tile plan: (checks, kind); kind 'F' = fused (ACT half + custom half),
# 'A' = ACT-all (exact tanh everywhere, level-1 is a DVE mul).  Two small
# starter tiles let compute begin ~1.5us earlier; the ratio (3072 F /
# 5120 A checks) balances ACT (~48us) against DVE (~46us).
TILE_PLAN = [(256, "F"), (256, "F"), (512, "F"), (512, "A"),
             (512, "A"), (512, "A"), (1024, "F"), (1024, "A"),
             (1024, "F"), (1024, "A"), (1024, "A"), (512, "A")]
OBS_AFTER = 2               # emit the obs tile after this many check tiles:
                            # its DVE chain fills the Vector stall that
                            # otherwise opens during the first big A-tile's
                            # 7.3us ACT activation
assert sum(c for c, _ in TILE_PLAN) == M
NTILES = len(TILE_PLAN)

# tanh(z) ~= z'(PA + u(PB + u)), z' = ALPHA*z, u = z'^2, for |z| <= ZCLIP
PA, PB, ALPHA, ZCLIP = 2.331135, -2.379626, 0.374835, 3.0
# ln(1+y) ~= y + y^2*(L0 + y*L1)
L0, L1_ = -0.5, 1.0 / 3.0

# Pool/GpSimd is deliberately UNUSED for tensor work: its software muls
# monopolize SBUF bandwidth and halve DVE+ACT throughput while active
# (measured v3: every DVE/ACT instr overlapping a Pool mul ran ~2x slow).
NACC = NTILES + 1           # accumulator columns (check tiles + obs)

_CACHE = {}


def _register_custom_ops():
    """Register the kernel-specific custom DVE ops with the dve_ops
    registry (the documented extension point is appending to OPS; doing it
    at runtime keeps kernel.py self-contained).  Pins uops_sha from the
    actual lowering so DveOp.compile's drift check passes."""
    if "ops" in _CACHE:
        return _CACHE["ops"]
    import concourse.dve_ops as dve_ops
    from concourse.dve_spec import (
        Spec, Src0, Src1, C0, C1, Zero, sq, lower, _has_src1,
    )
    from concourse.dve_uop import DveOpSpec
    from operator import add

    u = sq(Src0)
    body_t = (Src0 * (C0 + u * (C1 + u))) * Src1

    def ref_t(in0, in1, s0, s1, imm2):
        x = in0.astype(np.float32)
        uu = x * x
        return (x * (s0 + uu * (s1 + uu)) * in1.astype(np.float32)).astype(
            np.float32)

    tanh5m = dve_ops.DveOp(
        "TANH5M_ANT", Spec(body=body_t, reference=ref_t),
        subdim=False, uops_sha={})

    m = Src0 * Src1
    body_l = m + sq(m) * (C0 + m * C1)

    def ref_l(in0, in1, c0, c1, c2):
        y = in0.astype(np.float32) * in1.astype(np.float32)
        b = (y + y * y * (c0 + y * c1)).astype(np.float32)
        return b, b.reshape(b.shape[0], -1).sum(axis=-1, keepdims=True)

    log1pm = dve_ops.DveOp(
        "LOG1PM_ANT",
        Spec(body=body_l, accum=add, accum_init=Zero, reference=ref_l),
        subdim=False, uops_sha={})

    for op in (tanh5m, log1pm):
        if op.name not in dve_ops._SUB_OPCODE_FOR_NAME:
            dve_ops.OPS.append(op)
            dve_ops.CUSTOM_DVE_SPECS[op.name] = op.spec
            dve_ops._SUB_OPCODE_FOR_NAME[op.name] = (
                dve_ops._CUSTOM_DVE_ROW_BASE + len(dve_ops.OPS) - 1)
        shas = {}
        for ver in ("v3", "v4"):
            spec = DveOpSpec(
                name=op.name,
                opcode=dve_ops.get_dve_sub_opcode(op.name),
                uops=lower(op.spec, ver=ver),
                rd1_en=_has_src1(op.spec),
            )
            shas[ver] = spec.sha(ver)
        object.__setattr__(op, "uops_sha", shas)
    _CACHE["ops"] = (tanh5m, log1pm)
    return _CACHE["ops"]


def build_nc():
    import concourse.bacc as bacc
    import concourse.mybir as mybir
    import concourse.tile as tile
    from contextlib import ExitStack

    tanh5m, log1pm = _register_custom_ops()

    nc = bacc.Bacc("TRN2", target_bir_lowering=False, debug=False)
    f32 = mybir.dt.float32
    bf16 = mybir.dt.bfloat16
    f8 = mybir.dt.float8e4

    g_dram = nc.dram_tensor("g", [BL, TOT_SLOTS], f8, kind="ExternalInput")
    out = nc.dram_tensor("out", [128, NACC], f32, kind="ExternalOutput")

    Tanh = mybir.ActivationFunctionType.Tanh

    with tile.TileContext(nc) as tc:
        with ExitStack() as ctx:
            singles = ctx.enter_context(tc.tile_pool(name="singles", bufs=1))
            gp = ctx.enter_context(tc.tile_pool(name="gp", bufs=4))
            tp = ctx.enter_context(tc.tile_pool(name="tp", bufs=2))
            hp = ctx.enter_context(tc.tile_pool(name="hp", bufs=2))
            qp = ctx.enter_context(tc.tile_pool(name="qp", bufs=2))

            acc = singles.tile([128, NACC], f32)

            def check_tile(t, off, ct, kind):
                ts_ = ct * WC
                g = gp.tile([128, ts_], f8, tag=f"g{ts_}")
                nc.sync.dma_start(g[:], g_dram[:, off:off + ts_])
                h = hp.tile([128, ts_ // 2], bf16, tag=f"h{ts_}")
                if kind == "F":
                    tA = tp.tile([128, ts_ // 2], bf16, tag=f"tA{ts_}")
                    nc.scalar.activation(tA[:], g[:, 0:ts_ // 2], Tanh,
                                         bias=0.0, scale=1.0 / ALPHA)
                    nc.vector._custom_dve(
                        tanh5m, out=h[:], in0=g[:, ts_ // 2:ts_], in1=tA[:],
                        s0=PA, s1=PB)
                else:
                    tA = tp.tile([128, ts_], bf16, tag=f"tF{ts_}")
                    nc.scalar.activation(tA[:], g[:], Tanh,
                                         bias=0.0, scale=1.0 / ALPHA)
                    nc.vector.tensor_mul(h[:], tA[:, 0:ts_ // 2],
                                         tA[:, ts_ // 2:ts_])
                q = qp.tile([128, ts_ // 4], bf16, tag=f"q{ts_}")
                nc.vector.tensor_mul(q[:], h[:, 0:ts_ // 4],
                                     h[:, ts_ // 4:ts_ // 2])
                lnj = qp.tile([128, ct], bf16, tag=f"lnj{ts_}")
                nc.vector._custom_dve(
                    log1pm, out=lnj[:], in0=q[:, 0:ct], in1=q[:, ct:2 * ct],
                    s0=L0, s1=L1_, accum_out=acc[:, t:t + 1])

            def obs_tile():
                ob = M * WC
                g = gp.tile([128, OBS_SLOTS], f8, tag="go")
                nc.sync.dma_start(g[:], g_dram[:, ob:ob + OBS_SLOTS])
                tA = tp.tile([128, 512], bf16, tag="tAo")
                nc.scalar.activation(tA[:], g[:, 0:512], Tanh,
                                     bias=0.0, scale=1.0 / ALPHA)
                h = hp.tile([128, 512], bf16, tag="ho")
                nc.vector._custom_dve(
                    tanh5m, out=h[:], in0=g[:, 512:1024], in1=tA[:],
                    s0=PA, s1=PB)
                w = 512
                while w > 2 * K:
                    w //= 2
                    nh = qp.tile([128, w], bf16, tag=f"o{w}")
                    nc.vector.tensor_mul(nh[:], h[:, 0:w], h[:, w:2 * w])
                    h = nh
                lno = qp.tile([128, K], bf16, tag="lno")
                nc.vector._custom_dve(
                    log1pm, out=lno[:], in0=h[:, 0:K], in1=h[:, K:2 * K],
                    s0=L0, s1=L1_, accum_out=acc[:, NTILES:NTILES + 1])

            offs = np.cumsum([0] + [c * WC for c, _ in TILE_PLAN])
            for t in range(NTILES):
                check_tile(t, int(offs[t]), *TILE_PLAN[t])
                if t == OBS_AFTER:
                    obs_tile()

            nc.sync.dma_start(out[:, :], acc[:])

    nc.compile()
    return nc


def get_nc():
    if "nc" not in _CACHE:
        _CACHE["nc"] = build_nc()
    return _CACHE["nc"]


def build_slots(chk_idx, obs_idx):
    """Column j of the shipped tensor holds z'[idx] for slot-order:
    check tile t (ct checks at check-offset r0, slot-offset o = 8*r0):
                  A-half j = o + v*ct + c      -> chk[r0+c, v]
                  B-half j = o + 4*ct + v*ct + c -> chk[r0+c, v+4]
    obs tile:     A-half j = M*WC + v*K + k      -> obs[k, v]
                  B-half j = M*WC + 512 + v*K + k -> obs[k, v+64]
    so every tree level multiplies contiguous halves."""
    chk = np.asarray(chk_idx)
    obs = np.asarray(obs_idx)
    parts = []
    r0 = 0
    for ct, _ in TILE_PLAN:
        sub = chk[r0:r0 + ct]                           # [ct, 8]
        parts.append(sub[:, 0:4].T.reshape(-1))         # v-major A half
        parts.append(sub[:, 4:8].T.reshape(-1))         # v-major B half
        r0 += ct
    parts.append(obs[:, 0:WO // 2].T.reshape(-1))
    parts.append(obs[:, WO // 2:WO].T.reshape(-1))
    return np.concatenate(parts).astype(np.int64)


def make_in_maps(llrs, syndromes, observables, chk_idx, obs_idx):
    zp = (np.clip(np.asarray(llrs) * 0.5, -ZCLIP, ZCLIP) * ALPHA).astype(
        ml_dtypes.float8_e4m3)
    slots = build_slots(chk_idx, obs_idx)
    g_all = np.take(zp, slots, axis=1)                  # [B, TOT_SLOTS]
    # fold s = (1-2y) into the sign bit of the w=0 slot of each check
    v = g_all.view(np.uint8)
    syn = np.asarray(syndromes)
    r0 = 0
    for ct, _ in TILE_PLAN:
        cols = slice(8 * r0, 8 * r0 + ct)               # v=0 of A half
        v[:, cols] ^= (syn[:, r0:r0 + ct] != 0).astype(np.uint8) << 7
        r0 += ct
    yobs = (np.asarray(observables) != 0).astype(np.uint8) << 7
    v[:, M * WC:M * WC + K] ^= yobs                     # v=0 of obs A half
    return [{"g": g_all[BL * c:BL * (c + 1)]} for c in range(NCORES)]


def finish(results):
    total = 0.0
    for r in results:
        total += float(np.asarray(r["out"]).astype(np.float64).sum())
    loss = 0.5 * (M + K) * math.log(2.0) - 0.5 * total / B
    return np.float32(loss)


def kernel(llrs, syndromes, observables, chk_idx, obs_idx):
    from concourse.bass_utils import run_bass_kernel_spmd

    in_maps = make_in_maps(llrs, syndromes, observables, chk_idx, obs_idx)
    nc = get_nc()
    res = run_bass_kernel_spmd(nc, in_maps, core_ids=list(range(NCORES)))
    return finish(res.results)
